# revision 1
# baseline (speedup 1.0000x reference)
"""Trainium2 Bass kernel for KMeans assignment (argmin over centroid distances).

Problem: x [131072, 768] f32, centroids [768, 2000] f32
Output:  argmin_k ||x_n - c_k||^2  -> int32 [131072]

Math: argmin_k(||x||^2 - 2 x.c_k + ||c_k||^2) = argmax_k(x.c_k - 0.5||c_k||^2).
Per-core (data-parallel over 8 cores, 16384 rows each):
  - keep centroids resident in SBUF (bf16 hi/lo split for near-fp32 matmul
    precision at bf16 PE rate: x.c = xh.ch + xh.cl + xl.ch, dropping xl.cl)
  - per 128-row tile: DMA x, cast hi/lo, PE-transpose to [d, n] weights,
    matmul-accumulate scores into PSUM, DVE adds bias (-0.5||c||^2, broadcast
    from host), DVE max/max_index gives argmax along free axis.
"""

import os
import sys

for _p in ("/opt/trn_rl_repo",):
    if _p not in sys.path and os.path.isdir(_p):
        sys.path.insert(0, _p)

from contextlib import ExitStack

import numpy as np

import concourse.bass as bass
import concourse.tile as tile
from concourse import bacc, mybir
from concourse.bass_utils import run_bass_kernel_spmd

try:
    import ml_dtypes

    BF16 = np.dtype(ml_dtypes.bfloat16)
except ImportError:  # pragma: no cover
    BF16 = None

N, D, K = 131072, 768, 2000
NCORES = 8
NSH = N // NCORES  # 16384 rows per core
P = 128
DT = D // P  # 6 contraction tiles
# score chunks, each within one PSUM bank (<=512 fp32)
KOFF = [0, 512, 1024, 1536]
KW = [512, 512, 512, 464]
NB = 4

F32 = mybir.dt.float32
BF = mybir.dt.bfloat16
U32 = mybir.dt.uint32


def build_nc_screen(n_rows: int = NSH):
    """Phase-1 screening program: single bf16 matmul pass.

    Bias (-0.5||c||^2) is folded into the matmul as two extra contraction
    rows (ones-weights x [bias_hi; bias_lo]) so the vector engine only runs
    max/max_index. Outputs the argmax index and the top-2 score values per
    row; rows with a small top-2 margin get recomputed exactly in phase 2.
    """
    assert n_rows % P == 0
    nt = n_rows // P
    nc = bacc.Bacc("TRN2", target_bir_lowering=False, debug=False)

    x = nc.dram_tensor("x", [n_rows, D], F32, kind="ExternalInput").ap()
    c_in = nc.dram_tensor("c", [D, K], BF, kind="ExternalInput").ap()
    bias2 = nc.dram_tensor("bias2", [2, K], BF, kind="ExternalInput").ap()
    ones = nc.dram_tensor("ones", [2, P], BF, kind="ExternalInput").ap()
    ident = nc.dram_tensor("ident", [P, P], BF, kind="ExternalInput").ap()
    out = nc.dram_tensor("out", [n_rows, 1], U32, kind="ExternalOutput").ap()
    vals = nc.dram_tensor("vals", [n_rows, 2], F32, kind="ExternalOutput").ap()

    with tile.TileContext(nc) as tc, ExitStack() as ctx:
        const = ctx.enter_context(tc.tile_pool(name="const", bufs=1))
        xin_p = ctx.enter_context(tc.tile_pool(name="xin", bufs=3))
        xcast_p = ctx.enter_context(tc.tile_pool(name="xcast", bufs=2))
        xtp_p = ctx.enter_context(tc.tile_pool(name="xtp", bufs=1, space="PSUM"))
        xts_p = ctx.enter_context(tc.tile_pool(name="xts", bufs=2))
        sc_p = ctx.enter_context(tc.tile_pool(name="sc", bufs=1, space="PSUM"))
        ss_p = ctx.enter_context(tc.tile_pool(name="ss", bufs=2))
        mx_p = ctx.enter_context(tc.tile_pool(name="mx", bufs=4))

        c3 = c_in.rearrange("(t p) k -> t p k", p=P)
        c_tiles = []
        for d in range(DT):
            ct = const.tile([P, K], BF, tag=f"c_{d}")
            nc.sync.dma_start(ct[:], c3[d])
            c_tiles.append(ct)
        bias_t = const.tile([2, K], BF, tag="bias2")
        nc.sync.dma_start(bias_t[:], bias2[:, :])
        ones_t = const.tile([2, P], BF, tag="ones")
        nc.sync.dma_start(ones_t[:], ones[:, :])
        id_t = const.tile([P, P], BF, tag="ident")
        nc.sync.dma_start(id_t[:], ident[:, :])

        for t in range(nt):
            xin = xin_p.tile([P, D], F32)
            nc.scalar.dma_start(xin[:], x[t * P:(t + 1) * P, :])
            xh = xcast_p.tile([P, D], BF, tag="xh")
            nc.scalar.copy(xh[:], xin[:])
            xtp = xtp_p.tile([P, D], BF)
            for d in range(DT):
                nc.tensor.transpose(
                    xtp[:, d * P:(d + 1) * P], xh[:, d * P:(d + 1) * P], id_t[:])
            xts = xts_p.tile([P, D], BF)
            nc.scalar.copy(xts[:], xtp[:])

            banks = []
            for b in range(NB):
                # first three banks double-buffered: they are what the next
                # tile's first matmuls wait on (PSUM budget: 2+2+2+1 + 1 xtp)
                bank_tile = sc_p.tile([P, KW[b]], F32, tag=f"b{b}",
                                      name=f"bank{b}", bufs=(2 if b < 3 else 1))
                banks.append(bank_tile)
            # bias rows first so each bank's accumulation closes on d == DT-1
            for b in range(NB):
                nc.tensor.matmul(
                    banks[b][:], ones_t[:],
                    bias_t[:, KOFF[b]:KOFF[b] + KW[b]],
                    start=True, stop=False)
            for d in range(DT):
                for b in range(NB):
                    nc.tensor.matmul(
                        banks[b][:], xts[:, d * P:(d + 1) * P],
                        c_tiles[d][:, KOFF[b]:KOFF[b] + KW[b]],
                        start=False, stop=(d == DT - 1))

            ss = ss_p.tile([P, K], F32)
            for b in range(NB):
                nc.scalar.copy(ss[:, KOFF[b]:KOFF[b] + KW[b]], banks[b][:])

            mxv = mx_p.tile([P, 8], F32, tag="mxv")
            nc.vector.max(mxv[:], ss[:])
            mxi = mx_p.tile([P, 8], U32, tag="mxi")
            nc.vector.max_index(mxi[:], mxv[:], ss[:])
            nc.scalar.dma_start(out[t * P:(t + 1) * P, :], mxi[:, 0:1])
            nc.scalar.dma_start(vals[t * P:(t + 1) * P, :], mxv[:, 0:2])

    nc.compile()
    return nc


def build_nc(mode: str = "bf16x3", n_rows: int = NSH):
    """Build + compile the per-core Bass program.

    mode: 'bf16x3' (hi/lo split, 3 bf16 passes), 'fp32', 'fp32r', 'bf16'
    """
    assert n_rows % P == 0
    nt = n_rows // P
    nc = bacc.Bacc("TRN2", target_bir_lowering=False, debug=False)

    x = nc.dram_tensor("x", [n_rows, D], F32, kind="ExternalInput").ap()
    bias = nc.dram_tensor("bias", [P, K], F32, kind="ExternalInput").ap()
    out = nc.dram_tensor("out", [n_rows, 1], U32, kind="ExternalOutput").ap()

    split = mode == "bf16x3"
    cdt = BF if mode in ("bf16x3", "bf16") else F32
    mmdt = {"bf16x3": BF, "bf16": BF, "fp32": F32, "fp32r": mybir.dt.float32r}[mode]

    if split:
        c_hi = nc.dram_tensor("c_hi", [D, K], BF, kind="ExternalInput").ap()
        c_lo = nc.dram_tensor("c_lo", [D, K], BF, kind="ExternalInput").ap()
        c_srcs = [c_hi, c_lo]
    else:
        c_full = nc.dram_tensor("c", [D, K], cdt, kind="ExternalInput").ap()
        c_srcs = [c_full]
    ident = nc.dram_tensor("ident", [P, P], mmdt if mmdt != mybir.dt.float32r else F32,
                           kind="ExternalInput").ap()

    with tile.TileContext(nc) as tc, ExitStack() as ctx:
        const = ctx.enter_context(tc.tile_pool(name="const", bufs=1))
        xin_p = ctx.enter_context(tc.tile_pool(name="xin", bufs=3))
        xcast_p = ctx.enter_context(tc.tile_pool(name="xcast", bufs=2))
        xtp_p = ctx.enter_context(tc.tile_pool(name="xtp", bufs=2, space="PSUM"))
        xts_p = ctx.enter_context(tc.tile_pool(name="xts", bufs=2))
        sc_p = ctx.enter_context(tc.tile_pool(name="sc", bufs=1, space="PSUM"))
        ss_p = ctx.enter_context(tc.tile_pool(name="ss", bufs=2))
        mx_p = ctx.enter_context(tc.tile_pool(name="mx", bufs=4))

        # centroids resident in SBUF: [DT][P, K] per source (hi/lo or single)
        c_tiles = []
        for si, csrc in enumerate(c_srcs):
            c3 = csrc.rearrange("(t p) k -> t p k", p=P)
            tiles = []
            for d in range(DT):
                ct = const.tile([P, K], cdt, tag=f"c{si}_{d}")
                nc.sync.dma_start(ct[:], c3[d])
                tiles.append(ct)
            c_tiles.append(tiles)

        bias_t = const.tile([P, K], F32, tag="bias")
        nc.sync.dma_start(bias_t[:], bias[:, :])
        id_t = const.tile([P, P], ident.dtype, tag="ident")
        nc.sync.dma_start(id_t[:], ident[:, :])

        for t in range(nt):
            xin = xin_p.tile([P, D], F32)
            nc.scalar.dma_start(xin[:], x[t * P:(t + 1) * P, :])

            if split:
                xh = xcast_p.tile([P, D], BF, tag="xh")
                nc.scalar.copy(xh[:], xin[:])
                xl = xcast_p.tile([P, D], BF, tag="xl")
                nc.vector.tensor_sub(xl[:], xin[:], xh[:])
                tsrc = [xh, xl]
            elif mode == "bf16":
                xh = xcast_p.tile([P, D], BF, tag="xh")
                nc.scalar.copy(xh[:], xin[:])
                tsrc = [xh]
            else:
                tsrc = [xin]

            # transpose x tiles -> [d, n] layout for matmul weights
            nsrc = len(tsrc)
            tdt = BF if cdt == BF else F32
            xtp = xtp_p.tile([P, D * nsrc], tdt)
            for si, xsrc in enumerate(tsrc):
                for d in range(DT):
                    nc.tensor.transpose(
                        xtp[:, si * D + d * P: si * D + (d + 1) * P],
                        xsrc[:, d * P:(d + 1) * P],
                        id_t[:],
                    )
            xts = xts_p.tile([P, D * nsrc], tdt)
            nc.scalar.copy(xts[:], xtp[:])

            def w(si, d):
                return xts[:, si * D + d * P: si * D + (d + 1) * P]

            banks = []
            for b in range(NB):
                bank_tile = sc_p.tile([P, KW[b]], F32, tag=f"b{b}", name=f"bank{b}")
                banks.append(bank_tile)
            if split:
                # accumulate xh.ch + xh.cl + xl.ch over d
                for d in range(DT):
                    for b in range(NB):
                        nc.tensor.matmul(
                            banks[b][:], w(0, d),
                            c_tiles[0][d][:, KOFF[b]:KOFF[b] + KW[b]],
                            start=(d == 0), stop=False)
                    for b in range(NB):
                        nc.tensor.matmul(
                            banks[b][:], w(0, d),
                            c_tiles[1][d][:, KOFF[b]:KOFF[b] + KW[b]],
                            start=False, stop=False)
                    for b in range(NB):
                        nc.tensor.matmul(
                            banks[b][:], w(1, d),
                            c_tiles[0][d][:, KOFF[b]:KOFF[b] + KW[b]],
                            start=False, stop=(d == DT - 1))
            else:
                for d in range(DT):
                    for b in range(NB):
                        lhs = w(0, d)
                        rhs = c_tiles[0][d][:, KOFF[b]:KOFF[b] + KW[b]]
                        if mode == "fp32r":
                            lhs = lhs.bitcast(mybir.dt.float32r)
                            rhs = rhs.bitcast(mybir.dt.float32r)
                        nc.tensor.matmul(banks[b][:], lhs, rhs,
                                         start=(d == 0), stop=(d == DT - 1))

            ss = ss_p.tile([P, K], F32)
            for b in range(NB):
                nc.vector.tensor_add(
                    ss[:, KOFF[b]:KOFF[b] + KW[b]], banks[b][:],
                    bias_t[:, KOFF[b]:KOFF[b] + KW[b]])

            mxv = mx_p.tile([P, 8], F32, tag="mxv")
            nc.vector.max(mxv[:], ss[:])
            mxi = mx_p.tile([P, 8], U32, tag="mxi")
            nc.vector.max_index(mxi[:], mxv[:], ss[:])
            nc.scalar.dma_start(out[t * P:(t + 1) * P, :], mxi[:, 0:1])

    nc.compile()
    return nc


def make_in_maps(x: np.ndarray, centroids: np.ndarray, mode: str = "bf16x3",
                 n_rows: int = NSH, n_cores: int = NCORES):
    x = np.ascontiguousarray(x, dtype=np.float32)
    c = np.ascontiguousarray(centroids, dtype=np.float32)
    c_norm = (c.astype(np.float64) ** 2).sum(axis=0)
    bias = np.broadcast_to((-0.5 * c_norm).astype(np.float32), (P, K)).copy()

    base = {"bias": bias}
    if mode == "bf16x3":
        c_hi = c.astype(BF16)
        c_lo = (c - c_hi.astype(np.float32)).astype(BF16)
        base["c_hi"] = c_hi
        base["c_lo"] = c_lo
        base["ident"] = np.eye(P, dtype=BF16)
    elif mode == "bf16":
        base["c"] = c.astype(BF16)
        base["ident"] = np.eye(P, dtype=BF16)
    else:
        base["c"] = c
        base["ident"] = np.eye(P, dtype=np.float32)

    in_maps = []
    for i in range(n_cores):
        m = dict(base)
        m["x"] = x[i * n_rows:(i + 1) * n_rows]
        in_maps.append(m)
    return in_maps


_NC_CACHE = {}
LAST_RESULTS = []  # (label, BassKernelResults) of the most recent kernel() call


def _run_spmd(nc, in_maps, label):
    kw = {}
    if os.environ.get("KMEANS_TRACE"):
        kw["trace"] = True
        kw["tmpdir"] = os.environ.get("KMEANS_TRACE_DIR", "/tmp/km_trace") + "_" + label
        os.makedirs(kw["tmpdir"], exist_ok=True)
    res = run_bass_kernel_spmd(nc, in_maps, core_ids=list(range(NCORES)), **kw)
    LAST_RESULTS.append((label, res))
    return res

# Phase-2 capacity: rows per core recomputed exactly. Margin threshold:
# empirical max bf16 score error on randn data is ~0.2; flag anything under
# 4x that. ~5% of rows get flagged at this threshold.
P2_ROWS = 1024
MARGIN_TH = None  # set below after calibration constant


def _cached_nc(key, builder):
    if key not in _NC_CACHE:
        _NC_CACHE[key] = builder()
    return _NC_CACHE[key]


def make_screen_in_maps(x: np.ndarray, centroids: np.ndarray,
                        n_rows: int = NSH, n_cores: int = NCORES):
    x = np.ascontiguousarray(x, dtype=np.float32)
    c = np.ascontiguousarray(centroids, dtype=np.float32)
    c_norm = (c.astype(np.float64) ** 2).sum(axis=0)
    bias = (-0.5 * c_norm).astype(np.float32)
    bias_hi = bias.astype(BF16)
    bias_lo = (bias - bias_hi.astype(np.float32)).astype(BF16)
    base = {
        "c": c.astype(BF16),
        "bias2": np.stack([bias_hi, bias_lo]),
        "ones": np.ones((2, P), dtype=BF16),
        "ident": np.eye(P, dtype=BF16),
    }
    in_maps = []
    for i in range(n_cores):
        m = dict(base)
        m["x"] = x[i * n_rows:(i + 1) * n_rows]
        in_maps.append(m)
    return in_maps


def _run_exact(x_rows: np.ndarray, centroids: np.ndarray, n_rows: int):
    """Run the exact (bf16x3) program on x_rows padded to n_rows*NCORES."""
    nc = _cached_nc(("bf16x3", n_rows), lambda: build_nc("bf16x3", n_rows))
    total = n_rows * NCORES
    xp = np.zeros((total, D), dtype=np.float32)
    xp[: len(x_rows)] = x_rows
    in_maps = make_in_maps(xp, centroids, mode="bf16x3", n_rows=n_rows)
    res = _run_spmd(nc, in_maps, "phase2")
    out = np.concatenate(
        [res.results[i]["out"].reshape(n_rows) for i in range(NCORES)])
    return out[: len(x_rows)], res


def kernel(x: np.ndarray, centroids: np.ndarray) -> np.ndarray:
    mode = os.environ.get("KMEANS_MODE", "hybrid")
    LAST_RESULTS.clear()
    x = np.asarray(x)
    centroids = np.asarray(centroids)

    if mode != "hybrid":
        nc = _cached_nc((mode, NSH), lambda: build_nc(mode=mode))
        in_maps = make_in_maps(x, centroids, mode=mode)
        res = _run_spmd(nc, in_maps, mode)
        parts = [res.results[i]["out"].reshape(NSH) for i in range(NCORES)]
        return np.concatenate(parts).astype(np.int32)

    # phase 1: bf16 screen with top-2 margins
    nc1 = _cached_nc(("screen", NSH), lambda: build_nc_screen(NSH))
    in_maps = make_screen_in_maps(x, centroids)
    res1 = _run_spmd(nc1, in_maps, "phase1")
    idx = np.concatenate(
        [res1.results[i]["out"].reshape(NSH) for i in range(NCORES)]
    ).astype(np.int32)
    vals = np.concatenate(
        [res1.results[i]["vals"].reshape(NSH, 2) for i in range(NCORES)])

    margin = vals[:, 0] - vals[:, 1]
    th = float(os.environ.get("KMEANS_MARGIN_TH", "0.8"))
    flagged = np.flatnonzero(margin < th)

    # phase 2: exact recompute of flagged rows; pick the smallest padded
    # program that covers the count, chunking in the (unexpected) overflow case
    sizes = [512, 1024, 1536, 2048]
    per_core = min((s for s in sizes if s * NCORES >= len(flagged)),
                   default=sizes[-1])
    cap = per_core * NCORES
    for s in range(0, len(flagged), cap):
        rows = flagged[s:s + cap]
        exact_idx, _ = _run_exact(x[rows], centroids, per_core)
        idx[rows] = exact_idx
    return idx



# revision 3
# speedup vs baseline: 1.5794x; 1.5794x over previous
"""Trainium2 Bass kernel for KMeans assignment (argmin over centroid distances).

Problem: x [131072, 768] f32, centroids [768, 2000] f32
Output:  argmin_k ||x_n - c_k||^2  -> int32 [131072]

Math: argmin_k(||x||^2 - 2 x.c_k + ||c_k||^2) = argmax_k(x.c_k - 0.5||c_k||^2).

Data-parallel over 8 cores (16384 rows each). Two phases:
  phase 1 (screen): single fp16 matmul pass. x is transposed+cast to fp16 on
    the HOST, so the device loop is pure matmul: per 128-row tile, 4 bias
    matmuls (2-row hi/lo fp16, folds -0.5||c||^2) + 24 score matmuls
    accumulate into 4 PSUM banks; ACT evicts to SBUF; DVE max8/find_index8
    produce argmax + top-2 margin. fp16 score error sigma ~0.006, so only
    rows with margin < MARGIN_TH (~0.5%) need exact recompute.
  phase 2 (exact): flagged rows recomputed with fp16 hi/lo 3-pass matmul
    (x.c = xh.ch + xh.cl + xl.ch), bias as 3-row fp16 ladder -> ~1e-5 error.
"""

import os
import sys

for _p in ("/opt/trn_rl_repo",):
    if _p not in sys.path and os.path.isdir(_p):
        sys.path.insert(0, _p)

from contextlib import ExitStack

import numpy as np

import concourse.bass as bass
import concourse.tile as tile
from concourse import bacc, mybir
from concourse.bass_utils import run_bass_kernel_spmd

N, D, K = 131072, 768, 2000
NCORES = 8
NSH = N // NCORES  # 16384 rows per core
P = 128
DT = D // P  # 6 contraction tiles
# score chunks, each within one PSUM bank (<=512 fp32)
KOFF = [0, 512, 1024, 1536]
KW = [512, 512, 512, 464]
NB = 4

F32 = mybir.dt.float32
F16 = mybir.dt.float16
U32 = mybir.dt.uint32

# fp16 screen margin threshold: empirical max |score err| is ~0.025 on randn
# data (sigma ~0.006); flag anything under ~4x that.
MARGIN_TH_DEFAULT = 0.1


def build_km(n_rows: int, exact: bool):
    """One-pass fp16 screen (exact=False) or 3-pass fp16 hi/lo (exact=True).

    Inputs are host-prepared: xT (and xTl for exact) is x transposed to
    [D, n_rows] fp16 so no on-device transpose/cast is needed; c in fp16
    (plus cl residual for exact); bias ladder rows fold -0.5||c||^2 into the
    PSUM accumulation via a tiny extra matmul.
    """
    assert n_rows % P == 0
    nt = n_rows // P
    CHUNK = min(2048, n_rows)
    assert n_rows % CHUNK == 0
    nchunks = n_rows // CHUNK
    tpc = CHUNK // P
    NBIAS = 3 if exact else 2

    nc = bacc.Bacc("TRN2", target_bir_lowering=False, debug=False)

    xT = nc.dram_tensor("xT", [D, n_rows], F16, kind="ExternalInput").ap()
    c_in = nc.dram_tensor("c", [D, K], F16, kind="ExternalInput").ap()
    bias2 = nc.dram_tensor("bias2", [NBIAS, K], F16, kind="ExternalInput").ap()
    ones = nc.dram_tensor("ones", [NBIAS, P], F16, kind="ExternalInput").ap()
    if exact:
        xTl = nc.dram_tensor("xTl", [D, n_rows], F16, kind="ExternalInput").ap()
        cl_in = nc.dram_tensor("cl", [D, K], F16, kind="ExternalInput").ap()
    out = nc.dram_tensor("out", [n_rows, 1], U32, kind="ExternalOutput").ap()
    vals = nc.dram_tensor("vals", [n_rows, 2], F32, kind="ExternalOutput").ap()

    with tile.TileContext(nc) as tc, ExitStack() as ctx:
        const = ctx.enter_context(tc.tile_pool(name="const", bufs=1))
        xsl_p = ctx.enter_context(tc.tile_pool(name="xsl", bufs=2))
        sc_p = ctx.enter_context(tc.tile_pool(name="sc", bufs=2, space="PSUM"))
        ss_p = ctx.enter_context(tc.tile_pool(name="ss", bufs=2))
        mx_p = ctx.enter_context(tc.tile_pool(name="mx", bufs=4))

        c3 = c_in.rearrange("(t p) k -> t p k", p=P)
        c_tiles = []
        for d in range(DT):
            ct = const.tile([P, K], F16, tag=f"c_{d}")
            nc.sync.dma_start(ct[:], c3[d])
            c_tiles.append(ct)
        if exact:
            cl3 = cl_in.rearrange("(t p) k -> t p k", p=P)
            cl_tiles = []
            for d in range(DT):
                ct = const.tile([P, K], F16, tag=f"cl_{d}")
                nc.sync.dma_start(ct[:], cl3[d])
                cl_tiles.append(ct)
        bias_t = const.tile([NBIAS, K], F16, tag="bias2")
        nc.sync.dma_start(bias_t[:], bias2[:, :])
        ones_t = const.tile([NBIAS, P], F16, tag="ones")
        nc.sync.dma_start(ones_t[:], ones[:, :])

        xT3 = xT.rearrange("(t p) n -> t p n", p=P)
        if exact:
            xTl3 = xTl.rearrange("(t p) n -> t p n", p=P)

        for ch in range(nchunks):
            c0 = ch * CHUNK
            slabs = []
            for d in range(DT):
                sl = xsl_p.tile([P, CHUNK], F16, tag=f"sl{d}")
                nc.sync.dma_start(sl[:], xT3[d, :, c0:c0 + CHUNK])
                slabs.append(sl)
            if exact:
                slabs_l = []
                for d in range(DT):
                    sl = xsl_p.tile([P, CHUNK], F16, tag=f"sll{d}")
                    nc.sync.dma_start(sl[:], xTl3[d, :, c0:c0 + CHUNK])
                    slabs_l.append(sl)

            for tt in range(tpc):
                off = tt * P
                banks = [
                    sc_p.tile([P, KW[b]], F32, tag=f"b{b}", name=f"bank{b}")
                    for b in range(NB)
                ]
                for b in range(NB):
                    nc.tensor.matmul(
                        banks[b][:], ones_t[:],
                        bias_t[:, KOFF[b]:KOFF[b] + KW[b]],
                        start=True, stop=False)
                if exact:
                    # xh.ch + xh.cl + xl.ch
                    passes = [(slabs, c_tiles), (slabs, cl_tiles),
                              (slabs_l, c_tiles)]
                else:
                    passes = [(slabs, c_tiles)]
                np_ = len(passes)
                for pi, (xs, cs) in enumerate(passes):
                    for d in range(DT):
                        last = pi == np_ - 1 and d == DT - 1
                        for b in range(NB):
                            nc.tensor.matmul(
                                banks[b][:], xs[d][:, off:off + P],
                                cs[d][:, KOFF[b]:KOFF[b] + KW[b]],
                                start=False, stop=last)

                ss = ss_p.tile([P, K], F32)
                for b in range(NB):
                    nc.scalar.copy(ss[:, KOFF[b]:KOFF[b] + KW[b]], banks[b][:])

                t = ch * tpc + tt
                mxv = mx_p.tile([P, 8], F32, tag="mxv")
                nc.vector.max(mxv[:], ss[:])
                mxi = mx_p.tile([P, 8], U32, tag="mxi")
                nc.vector.max_index(mxi[:], mxv[:], ss[:])
                nc.gpsimd.dma_start(out[t * P:(t + 1) * P, :], mxi[:, 0:1])
                nc.gpsimd.dma_start(vals[t * P:(t + 1) * P, :], mxv[:, 0:2])

    nc.compile()
    return nc


def _split16(a: np.ndarray, levels: int):
    """Split f32/f64 array into `levels` fp16 rows summing to ~a."""
    rows = []
    rem = a.astype(np.float64)
    for _ in range(levels):
        h = rem.astype(np.float16)
        rows.append(h)
        rem = rem - h.astype(np.float64)
    return rows


def _prep_c(centroids: np.ndarray):
    c = np.ascontiguousarray(centroids, dtype=np.float32)
    c_norm = (c.astype(np.float64) ** 2).sum(axis=0)
    bias = -0.5 * c_norm
    c16 = c.astype(np.float16)
    cl16 = (c - c16.astype(np.float32)).astype(np.float16)
    return c16, cl16, bias


def make_in_maps_screen(x: np.ndarray, c16, bias):
    b_rows = _split16(bias, 2)
    base = {
        "c": c16,
        "bias2": np.stack(b_rows),
        "ones": np.ones((2, P), np.float16),
    }
    in_maps = []
    for i in range(NCORES):
        m = dict(base)
        m["xT"] = np.ascontiguousarray(
            x[i * NSH:(i + 1) * NSH].T.astype(np.float16))
        in_maps.append(m)
    return in_maps


def make_in_maps_exact(x_rows: np.ndarray, c16, cl16, bias, n_rows: int):
    total = n_rows * NCORES
    xp = np.zeros((total, D), dtype=np.float32)
    xp[: len(x_rows)] = x_rows
    xh = xp.astype(np.float16)
    xl = (xp - xh.astype(np.float32)).astype(np.float16)
    b_rows = _split16(bias, 3)
    base = {
        "c": c16,
        "cl": cl16,
        "bias2": np.stack(b_rows),
        "ones": np.ones((3, P), np.float16),
    }
    in_maps = []
    for i in range(NCORES):
        m = dict(base)
        m["xT"] = np.ascontiguousarray(xh[i * n_rows:(i + 1) * n_rows].T)
        m["xTl"] = np.ascontiguousarray(xl[i * n_rows:(i + 1) * n_rows].T)
        in_maps.append(m)
    return in_maps


_NC_CACHE = {}
LAST_RESULTS = []  # (label, BassKernelResults) of the most recent kernel() call


def _cached_nc(key, builder):
    if key not in _NC_CACHE:
        _NC_CACHE[key] = builder()
    return _NC_CACHE[key]


def _run_spmd(nc, in_maps, label):
    kw = {}
    if os.environ.get("KMEANS_TRACE"):
        import shutil

        kw["trace"] = True
        kw["tmpdir"] = os.environ.get("KMEANS_TRACE_DIR", "/tmp/km_trace") + "_" + label
        shutil.rmtree(kw["tmpdir"], ignore_errors=True)
        os.makedirs(kw["tmpdir"], exist_ok=True)
    res = run_bass_kernel_spmd(nc, in_maps, core_ids=list(range(NCORES)), **kw)
    LAST_RESULTS.append((label, res))
    return res


def kernel(x: np.ndarray, centroids: np.ndarray) -> np.ndarray:
    LAST_RESULTS.clear()
    x = np.asarray(x)
    centroids = np.asarray(centroids)
    c16, cl16, bias = _prep_c(centroids)

    # phase 1: fp16 screen with top-2 margins
    nc1 = _cached_nc(("screen", NSH), lambda: build_km(NSH, exact=False))
    res1 = _run_spmd(nc1, make_in_maps_screen(x, c16, bias), "phase1")
    idx = np.concatenate(
        [res1.results[i]["out"].reshape(NSH) for i in range(NCORES)]
    ).astype(np.int32)
    vals = np.concatenate(
        [res1.results[i]["vals"].reshape(NSH, 2) for i in range(NCORES)])

    margin = vals[:, 0] - vals[:, 1]
    th = float(os.environ.get("KMEANS_MARGIN_TH", str(MARGIN_TH_DEFAULT)))
    flagged = np.flatnonzero(margin < th)

    # phase 2: exact recompute of flagged rows; smallest padded program that
    # covers the count, chunking in the (unexpected) overflow case
    sizes = [128, 256, 512, 1024, 2048]
    per_core = min((s for s in sizes if s * NCORES >= len(flagged)),
                   default=sizes[-1])
    cap = per_core * NCORES
    for s in range(0, len(flagged), cap):
        rows = flagged[s:s + cap]
        nc2 = _cached_nc(("exact", per_core),
                         lambda: build_km(per_core, exact=True))
        res2 = _run_spmd(
            nc2, make_in_maps_exact(x[rows], c16, cl16, bias, per_core),
            "phase2")
        exact_idx = np.concatenate(
            [res2.results[i]["out"].reshape(per_core) for i in range(NCORES)])
        idx[rows] = exact_idx[: len(rows)].astype(np.int32)
    return idx


# revision 5
# speedup vs baseline: 1.6576x; 1.0495x over previous
"""Trainium2 Bass kernel for KMeans assignment (argmin over centroid distances).

Problem: x [131072, 768] f32, centroids [768, 2000] f32
Output:  argmin_k ||x_n - c_k||^2  -> int32 [131072]

Math: argmin_k(||x||^2 - 2 x.c_k + ||c_k||^2) = argmax_k(x.c_k - 0.5||c_k||^2).

Data-parallel over 8 cores (16384 rows each). Two phases:
  phase 1 (screen): single fp16 matmul pass. x is transposed+cast to fp16 on
    the HOST, so the device loop is pure matmul: per 128-row tile, 4 bias
    matmuls (2-row hi/lo fp16, folds -0.5||c||^2) + 24 score matmuls
    accumulate into 4 PSUM banks; ACT evicts to SBUF; DVE max8/find_index8
    produce argmax + top-2 margin. fp16 score error sigma ~0.006, so only
    rows with margin < MARGIN_TH (~0.5%) need exact recompute.
  phase 2 (exact): flagged rows recomputed with fp16 hi/lo 3-pass matmul
    (x.c = xh.ch + xh.cl + xl.ch), bias as 3-row fp16 ladder -> ~1e-5 error.
"""

import os
import sys

for _p in ("/opt/trn_rl_repo",):
    if _p not in sys.path and os.path.isdir(_p):
        sys.path.insert(0, _p)

from contextlib import ExitStack

import numpy as np

import concourse.bass as bass
import concourse.tile as tile
from concourse import bacc, mybir
from concourse.bass_utils import run_bass_kernel_spmd

N, D, K = 131072, 768, 2000
NCORES = 8
NSH = N // NCORES  # 16384 rows per core
P = 128
DT = D // P  # 6 contraction tiles
# score chunks, each within one PSUM bank (<=512 fp32)
KOFF = [0, 512, 1024, 1536]
KW = [512, 512, 512, 464]
NB = 4

F32 = mybir.dt.float32
F16 = mybir.dt.float16
U32 = mybir.dt.uint32

# fp16 screen margin threshold: empirical score err sigma ~0.008, max ~0.05
# on randn data. 0.06 keeps sim misses at 0 while flagging ~0.6% of rows
# (fits the 128-rows/core phase-2 program).
MARGIN_TH_DEFAULT = 0.06


def build_km(n_rows: int, exact: bool):
    """One-pass fp16 screen (exact=False) or 3-pass fp16 hi/lo (exact=True).

    Inputs are host-prepared: xT (and xTl for exact) is x transposed to
    [D, n_rows] fp16 so no on-device transpose/cast is needed; c in fp16
    (plus cl residual for exact); bias ladder rows fold -0.5||c||^2 into the
    PSUM accumulation via a tiny extra matmul.
    """
    assert n_rows % P == 0
    nt = n_rows // P
    CHUNK = min(2048, n_rows)
    assert n_rows % CHUNK == 0
    nchunks = n_rows // CHUNK
    tpc = CHUNK // P
    NBIAS = 3 if exact else 2

    nc = bacc.Bacc("TRN2", target_bir_lowering=False, debug=False)

    xT = nc.dram_tensor("xT", [D, n_rows], F16, kind="ExternalInput").ap()
    c_in = nc.dram_tensor("c", [D, K], F16, kind="ExternalInput").ap()
    bias2 = nc.dram_tensor("bias2", [NBIAS, K], F16, kind="ExternalInput").ap()
    ones = nc.dram_tensor("ones", [NBIAS, P], F16, kind="ExternalInput").ap()
    if exact:
        xTl = nc.dram_tensor("xTl", [D, n_rows], F16, kind="ExternalInput").ap()
        cl_in = nc.dram_tensor("cl", [D, K], F16, kind="ExternalInput").ap()
    out = nc.dram_tensor("out", [n_rows, 1], U32, kind="ExternalOutput").ap()
    vals = nc.dram_tensor("vals", [n_rows, 2], F32, kind="ExternalOutput").ap()

    with tile.TileContext(nc) as tc, ExitStack() as ctx:
        const = ctx.enter_context(tc.tile_pool(name="const", bufs=1))
        xsl_p = ctx.enter_context(tc.tile_pool(name="xsl", bufs=2))
        sc_p = ctx.enter_context(tc.tile_pool(name="sc", bufs=2, space="PSUM"))
        ss_p = ctx.enter_context(tc.tile_pool(name="ss", bufs=2))
        mx_p = ctx.enter_context(tc.tile_pool(name="mx", bufs=4))

        # single 3D-AP DMA for all 6 contraction tiles of c (and cl)
        c3 = c_in.rearrange("(t p) k -> p t k", p=P)
        c_all = const.tile([P, DT, K], F16, tag="c_all")
        nc.sync.dma_start(c_all[:], c3[:, :, :])
        c_tiles = [c_all[:, d, :] for d in range(DT)]
        if exact:
            cl3 = cl_in.rearrange("(t p) k -> p t k", p=P)
            cl_all = const.tile([P, DT, K], F16, tag="cl_all")
            nc.sync.dma_start(cl_all[:], cl3[:, :, :])
            cl_tiles = [cl_all[:, d, :] for d in range(DT)]
        bias_t = const.tile([NBIAS, K], F16, tag="bias2")
        nc.scalar.dma_start(bias_t[:], bias2[:, :])
        ones_t = const.tile([NBIAS, P], F16, tag="ones")
        nc.scalar.dma_start(ones_t[:], ones[:, :])

        xT3 = xT.rearrange("(t p) n -> p t n", p=P)
        if exact:
            xTl3 = xTl.rearrange("(t p) n -> p t n", p=P)

        for ch in range(nchunks):
            c0 = ch * CHUNK
            slab = xsl_p.tile([P, DT, CHUNK], F16, tag="sl")
            nc.sync.dma_start(slab[:], xT3[:, :, c0:c0 + CHUNK])
            slabs = [slab[:, d, :] for d in range(DT)]
            if exact:
                slab_l = xsl_p.tile([P, DT, CHUNK], F16, tag="sll")
                nc.sync.dma_start(slab_l[:], xTl3[:, :, c0:c0 + CHUNK])
                slabs_l = [slab_l[:, d, :] for d in range(DT)]

            for tt in range(tpc):
                off = tt * P
                banks = [
                    sc_p.tile([P, KW[b]], F32, tag=f"b{b}", name=f"bank{b}")
                    for b in range(NB)
                ]
                for b in range(NB):
                    nc.tensor.matmul(
                        banks[b][:], ones_t[:],
                        bias_t[:, KOFF[b]:KOFF[b] + KW[b]],
                        start=True, stop=False)
                if exact:
                    # xh.ch + xh.cl + xl.ch
                    passes = [(slabs, c_tiles), (slabs, cl_tiles),
                              (slabs_l, c_tiles)]
                else:
                    passes = [(slabs, c_tiles)]
                np_ = len(passes)
                for pi, (xs, cs) in enumerate(passes):
                    for d in range(DT):
                        last = pi == np_ - 1 and d == DT - 1
                        for b in range(NB):
                            nc.tensor.matmul(
                                banks[b][:], xs[d][:, off:off + P],
                                cs[d][:, KOFF[b]:KOFF[b] + KW[b]],
                                start=False, stop=last)

                ss = ss_p.tile([P, K], F32)
                for b in range(NB):
                    nc.scalar.copy(ss[:, KOFF[b]:KOFF[b] + KW[b]], banks[b][:])

                t = ch * tpc + tt
                mxv = mx_p.tile([P, 8], F32, tag="mxv")
                nc.vector.max(mxv[:], ss[:])
                mxi = mx_p.tile([P, 8], U32, tag="mxi")
                nc.vector.max_index(mxi[:], mxv[:], ss[:])
                nc.gpsimd.dma_start(out[t * P:(t + 1) * P, :], mxi[:, 0:1])
                nc.gpsimd.dma_start(vals[t * P:(t + 1) * P, :], mxv[:, 0:2])

    nc.compile()
    return nc


def _split16(a: np.ndarray, levels: int):
    """Split f32/f64 array into `levels` fp16 rows summing to ~a."""
    rows = []
    rem = a.astype(np.float64)
    for _ in range(levels):
        h = rem.astype(np.float16)
        rows.append(h)
        rem = rem - h.astype(np.float64)
    return rows


def _prep_c(centroids: np.ndarray):
    c = np.ascontiguousarray(centroids, dtype=np.float32)
    c_norm = (c.astype(np.float64) ** 2).sum(axis=0)
    bias = -0.5 * c_norm
    c16 = c.astype(np.float16)
    cl16 = (c - c16.astype(np.float32)).astype(np.float16)
    return c16, cl16, bias


def make_in_maps_screen(x: np.ndarray, c16, bias):
    b_rows = _split16(bias, 2)
    base = {
        "c": c16,
        "bias2": np.stack(b_rows),
        "ones": np.ones((2, P), np.float16),
    }
    in_maps = []
    for i in range(NCORES):
        m = dict(base)
        m["xT"] = np.ascontiguousarray(
            x[i * NSH:(i + 1) * NSH].T.astype(np.float16))
        in_maps.append(m)
    return in_maps


def make_in_maps_exact(x_rows: np.ndarray, c16, cl16, bias, n_rows: int):
    total = n_rows * NCORES
    xp = np.zeros((total, D), dtype=np.float32)
    xp[: len(x_rows)] = x_rows
    xh = xp.astype(np.float16)
    xl = (xp - xh.astype(np.float32)).astype(np.float16)
    b_rows = _split16(bias, 3)
    base = {
        "c": c16,
        "cl": cl16,
        "bias2": np.stack(b_rows),
        "ones": np.ones((3, P), np.float16),
    }
    in_maps = []
    for i in range(NCORES):
        m = dict(base)
        m["xT"] = np.ascontiguousarray(xh[i * n_rows:(i + 1) * n_rows].T)
        m["xTl"] = np.ascontiguousarray(xl[i * n_rows:(i + 1) * n_rows].T)
        in_maps.append(m)
    return in_maps


_NC_CACHE = {}
LAST_RESULTS = []  # (label, BassKernelResults) of the most recent kernel() call


def _cached_nc(key, builder):
    if key not in _NC_CACHE:
        _NC_CACHE[key] = builder()
    return _NC_CACHE[key]


def _run_spmd(nc, in_maps, label):
    kw = {}
    if os.environ.get("KMEANS_TRACE"):
        import shutil

        kw["trace"] = True
        kw["tmpdir"] = os.environ.get("KMEANS_TRACE_DIR", "/tmp/km_trace") + "_" + label
        shutil.rmtree(kw["tmpdir"], ignore_errors=True)
        os.makedirs(kw["tmpdir"], exist_ok=True)
    res = run_bass_kernel_spmd(nc, in_maps, core_ids=list(range(NCORES)), **kw)
    LAST_RESULTS.append((label, res))
    return res


def kernel(x: np.ndarray, centroids: np.ndarray) -> np.ndarray:
    LAST_RESULTS.clear()
    x = np.asarray(x)
    centroids = np.asarray(centroids)
    c16, cl16, bias = _prep_c(centroids)

    # phase 1: fp16 screen with top-2 margins
    nc1 = _cached_nc(("screen", NSH), lambda: build_km(NSH, exact=False))
    res1 = _run_spmd(nc1, make_in_maps_screen(x, c16, bias), "phase1")
    idx = np.concatenate(
        [res1.results[i]["out"].reshape(NSH) for i in range(NCORES)]
    ).astype(np.int32)
    vals = np.concatenate(
        [res1.results[i]["vals"].reshape(NSH, 2) for i in range(NCORES)])

    margin = vals[:, 0] - vals[:, 1]
    th = float(os.environ.get("KMEANS_MARGIN_TH", str(MARGIN_TH_DEFAULT)))
    flagged = np.flatnonzero(margin < th)

    # phase 2: exact recompute of flagged rows; smallest padded program that
    # covers the count, chunking in the (unexpected) overflow case
    sizes = [128, 256, 512, 1024, 2048]
    per_core = min((s for s in sizes if s * NCORES >= len(flagged)),
                   default=sizes[-1])
    cap = per_core * NCORES
    for s in range(0, len(flagged), cap):
        rows = flagged[s:s + cap]
        nc2 = _cached_nc(("exact", per_core),
                         lambda: build_km(per_core, exact=True))
        res2 = _run_spmd(
            nc2, make_in_maps_exact(x[rows], c16, cl16, bias, per_core),
            "phase2")
        exact_idx = np.concatenate(
            [res2.results[i]["out"].reshape(per_core) for i in range(NCORES)])
        idx[rows] = exact_idx[: len(rows)].astype(np.int32)
    return idx


# revision 9
# speedup vs baseline: 1.8002x; 1.0860x over previous
"""Trainium2 Bass kernel for KMeans assignment (argmin over centroid distances).

Problem: x [131072, 768] f32, centroids [768, 2000] f32
Output:  argmin_k ||x_n - c_k||^2  -> int32 [131072]

Math: argmin_k(||x||^2 - 2 x.c_k + ||c_k||^2) = argmax_k(x.c_k - 0.5||c_k||^2).

Data-parallel over 8 cores (16384 rows each). Two phases:
  phase 1 (screen): single fp16 matmul pass. x is transposed+cast to fp16 on
    the HOST, so the device loop is pure matmul: per 128-row tile, 4 bias
    matmuls (2-row hi/lo fp16, folds -0.5||c||^2) + 24 score matmuls
    accumulate into 4 PSUM banks; ACT evicts to SBUF; DVE max8/find_index8
    produce argmax + top-2 margin. fp16 score error sigma ~0.006, so only
    rows with margin < MARGIN_TH (~0.5%) need exact recompute.
  phase 2 (exact): flagged rows recomputed with fp16 hi/lo 3-pass matmul
    (x.c = xh.ch + xh.cl + xl.ch), bias as 3-row fp16 ladder -> ~1e-5 error.
"""

import os
import sys

for _p in ("/opt/trn_rl_repo",):
    if _p not in sys.path and os.path.isdir(_p):
        sys.path.insert(0, _p)

from contextlib import ExitStack

import numpy as np

import concourse.bass as bass
import concourse.tile as tile
from concourse import bacc, mybir
from concourse.bass_utils import run_bass_kernel_spmd

N, D, K = 131072, 768, 2000
NCORES = 8
NSH = N // NCORES  # 16384 rows per core
P = 128
DT = D // P  # 6 contraction tiles
# score chunks, each within one PSUM bank (<=512 fp32)
KOFF = [0, 512, 1024, 1536]
KW = [512, 512, 512, 464]
NB = 4

F32 = mybir.dt.float32
F16 = mybir.dt.float16
U32 = mybir.dt.uint32

# fp16 screen margin threshold: empirical score err sigma ~0.008, max ~0.05
# on randn data. 0.06 keeps sim misses at 0 while flagging ~0.6% of rows
# (fits the 128-rows/core phase-2 program).
MARGIN_TH_DEFAULT = 0.06


def build_km(n_rows: int, exact: bool):
    """One-pass fp16 screen (exact=False) or 3-pass fp16 hi/lo (exact=True).

    Inputs are host-prepared: xT (and xTl for exact) is x transposed to
    [D, n_rows] fp16 so no on-device transpose/cast is needed; c in fp16
    (plus cl residual for exact); bias ladder rows fold -0.5||c||^2 into the
    PSUM accumulation via a tiny extra matmul.
    """
    assert n_rows % P == 0
    nt = n_rows // P
    CHUNK = min(2048, n_rows)
    assert n_rows % CHUNK == 0
    nchunks = n_rows // CHUNK
    tpc = CHUNK // P
    NBIAS = 3 if exact else 2

    nc = bacc.Bacc("TRN2", target_bir_lowering=False, debug=False)

    xT = nc.dram_tensor("xT", [D, n_rows], F16, kind="ExternalInput").ap()
    c_in = nc.dram_tensor("c", [D, K], F16, kind="ExternalInput").ap()
    bias2 = nc.dram_tensor("bias2", [NBIAS, K], F16, kind="ExternalInput").ap()
    ones = nc.dram_tensor("ones", [NBIAS, P], F16, kind="ExternalInput").ap()
    if exact:
        xTl = nc.dram_tensor("xTl", [D, n_rows], F16, kind="ExternalInput").ap()
        cl_in = nc.dram_tensor("cl", [D, K], F16, kind="ExternalInput").ap()
    out = nc.dram_tensor("out", [n_rows, 1], U32, kind="ExternalOutput").ap()
    vals = nc.dram_tensor("vals", [n_rows, 2], F32, kind="ExternalOutput").ap()

    with tile.TileContext(nc) as tc, ExitStack() as ctx:
        const = ctx.enter_context(tc.tile_pool(name="const", bufs=1))
        xsl_p = ctx.enter_context(tc.tile_pool(name="xsl", bufs=2))
        sc_p = ctx.enter_context(tc.tile_pool(name="sc", bufs=2, space="PSUM"))
        ss_p = ctx.enter_context(tc.tile_pool(name="ss", bufs=2))
        mx_p = ctx.enter_context(tc.tile_pool(name="mx", bufs=4))

        # single 3D-AP DMA for all 6 contraction tiles of c (and cl)
        c3 = c_in.rearrange("(t p) k -> p t k", p=P)
        c_all = const.tile([P, DT, K], F16, tag="c_all")
        nc.sync.dma_start(c_all[:], c3[:, :, :])
        c_tiles = [c_all[:, d, :] for d in range(DT)]
        if exact:
            cl3 = cl_in.rearrange("(t p) k -> p t k", p=P)
            cl_all = const.tile([P, DT, K], F16, tag="cl_all")
            nc.sync.dma_start(cl_all[:], cl3[:, :, :])
            cl_tiles = [cl_all[:, d, :] for d in range(DT)]
        bias_t = const.tile([NBIAS, K], F16, tag="bias2")
        nc.scalar.dma_start(bias_t[:], bias2[:, :])
        ones_t = const.tile([NBIAS, P], F16, tag="ones")
        nc.scalar.dma_start(ones_t[:], ones[:, :])

        xT3 = xT.rearrange("(t p) n -> p t n", p=P)
        if exact:
            xTl3 = xTl.rearrange("(t p) n -> p t n", p=P)

        for ch in range(nchunks):
            c0 = ch * CHUNK
            slab = xsl_p.tile([P, DT, CHUNK], F16, tag="sl")
            nc.sync.dma_start(slab[:], xT3[:, :, c0:c0 + CHUNK])
            slabs = [slab[:, d, :] for d in range(DT)]
            if exact:
                slab_l = xsl_p.tile([P, DT, CHUNK], F16, tag="sll")
                nc.sync.dma_start(slab_l[:], xTl3[:, :, c0:c0 + CHUNK])
                slabs_l = [slab_l[:, d, :] for d in range(DT)]

            for tt in range(tpc):
                off = tt * P
                banks = [
                    sc_p.tile([P, KW[b]], F32, tag=f"b{b}", name=f"bank{b}")
                    for b in range(NB)
                ]
                for b in range(NB):
                    nc.tensor.matmul(
                        banks[b][:], ones_t[:],
                        bias_t[:, KOFF[b]:KOFF[b] + KW[b]],
                        start=True, stop=False)
                if exact:
                    # xh.ch + xh.cl + xl.ch
                    passes = [(slabs, c_tiles), (slabs, cl_tiles),
                              (slabs_l, c_tiles)]
                else:
                    passes = [(slabs, c_tiles)]
                np_ = len(passes)
                for pi, (xs, cs) in enumerate(passes):
                    for d in range(DT):
                        last = pi == np_ - 1 and d == DT - 1
                        for b in range(NB):
                            nc.tensor.matmul(
                                banks[b][:], xs[d][:, off:off + P],
                                cs[d][:, KOFF[b]:KOFF[b] + KW[b]],
                                start=False, stop=last)

                ss = ss_p.tile([P, K], F32)
                for b in range(NB):
                    nc.scalar.copy(ss[:, KOFF[b]:KOFF[b] + KW[b]], banks[b][:])

                t = ch * tpc + tt
                mxv = mx_p.tile([P, 8], F32, tag="mxv")
                nc.vector.max(mxv[:], ss[:])
                mxi = mx_p.tile([P, 8], U32, tag="mxi")
                nc.vector.max_index(mxi[:], mxv[:], ss[:])
                nc.gpsimd.dma_start(out[t * P:(t + 1) * P, :], mxi[:, 0:1])
                nc.gpsimd.dma_start(vals[t * P:(t + 1) * P, :], mxv[:, 0:2])

    nc.compile()
    return nc


#  Orientation-B screen: scores computed k-on-partitions so the -0.5||c||^2
#  bias is a per-partition ACT bias (no bias matmuls at all). Each kt-tile's
#  [128k, 512n] PSUM block is evicted by ACT as u = BETA*s + bias_k + C0,
#  pinned inside the f32 octave [256, 512) so ulp is constant 2^-15. DVE
#  packs the kt-tile id into the low 4 mantissa bits ((u & ~0xF) | (15-kt))
#  and chain-maxes the 16 kt-tiles; PE transposes the 128-wide survivor per
#  row-tile; max8 + find_index8 (128-wide) give the winning partition p and
#  packed value. Host decodes k = (15 - (bits & 0xF))*128 + p. Ties at the
#  2^-11 quantum prefer smaller k (matching first-argmin) and near-ties are
#  margin-flagged for phase 2 anyway.
BETA = 0.25
C0 = 448.0
K2 = 2048  # K padded to 16 full kt-tiles; pads get bias -> u=257, never win
KT = K2 // P
WIN = 512  # rows per score window (one PSUM bank of [128k, 512n])


def build_km_b(n_rows: int):
    assert n_rows % P == 0
    CHUNK = min(2048, n_rows)
    assert n_rows % CHUNK == 0
    nchunks = n_rows // CHUNK
    wpc = CHUNK // WIN

    nc = bacc.Bacc("TRN2", target_bir_lowering=False, debug=False)

    xT = nc.dram_tensor("xT", [D, n_rows], F16, kind="ExternalInput").ap()
    c_in = nc.dram_tensor("c", [D, K2], F16, kind="ExternalInput").ap()
    biasv = nc.dram_tensor("biasv", [P, KT], F32, kind="ExternalInput").ap()
    ident = nc.dram_tensor("ident", [P, P], F32, kind="ExternalInput").ap()
    out = nc.dram_tensor("out", [n_rows, 1], U32, kind="ExternalOutput").ap()
    vals = nc.dram_tensor("vals", [n_rows, 2], F32, kind="ExternalOutput").ap()

    with tile.TileContext(nc) as tc, ExitStack() as ctx:
        const = ctx.enter_context(tc.tile_pool(name="const", bufs=1))
        xsl_p = ctx.enter_context(tc.tile_pool(name="xsl", bufs=2))
        ps_p = ctx.enter_context(tc.tile_pool(name="ps", bufs=4, space="PSUM"))
        tp_p = ctx.enter_context(tc.tile_pool(name="tp", bufs=2, space="PSUM"))
        ev_p = ctx.enter_context(tc.tile_pool(name="ev", bufs=3))
        pk_p = ctx.enter_context(tc.tile_pool(name="pk", bufs=2))
        acc_p = ctx.enter_context(tc.tile_pool(name="acc", bufs=2))
        tps_p = ctx.enter_context(tc.tile_pool(name="tps", bufs=2))
        mx_p = ctx.enter_context(tc.tile_pool(name="mx", bufs=4))

        c3 = c_in.rearrange("(t p) k -> p t k", p=P)
        c_all = const.tile([P, DT, K2], F16, tag="c_all")
        nc.gpsimd.dma_start(c_all[:], c3[:, :, :])
        biasv_t = const.tile([P, KT], F32, tag="biasv")
        nc.scalar.dma_start(biasv_t[:], biasv[:, :])
        ident_t = const.tile([P, P], F32, tag="ident")
        nc.scalar.dma_start(ident_t[:], ident[:, :])

        xT3 = xT.rearrange("(t p) n -> p t n", p=P)
        ID = mybir.ActivationFunctionType.Identity
        for ch in range(nchunks):
            c0 = ch * CHUNK
            slab = xsl_p.tile([P, DT, CHUNK], F16, tag="sl")
            nc.sync.dma_start(slab[:], xT3[:, :, c0:c0 + CHUNK])
            for w in range(wpc):
                n0 = w * WIN
                acc = None
                for kt in range(KT):
                    ps = ps_p.tile([P, WIN], F32, tag="ps", name="psb")
                    for d in range(DT):
                        nc.tensor.matmul(
                            ps[:], c_all[:, d, kt * P:(kt + 1) * P],
                            slab[:, d, n0:n0 + WIN],
                            start=(d == 0), stop=(d == DT - 1))
                    sb = ev_p.tile([P, WIN], F32, tag="ev")
                    nc.scalar.activation(sb[:], ps[:], ID,
                                         bias=biasv_t[:, kt:kt + 1],
                                         scale=BETA)
                    pk = pk_p.tile([P, WIN], F32, tag="pk")
                    nc.vector.tensor_scalar(
                        pk.bitcast(U32)[:], sb.bitcast(U32)[:],
                        0xFFFFFFF0, KT - 1 - kt,
                        op0=mybir.AluOpType.bitwise_and,
                        op1=mybir.AluOpType.bitwise_or)
                    if acc is None:
                        acc = pk
                    else:
                        nacc = acc_p.tile([P, WIN], F32, tag="acc")
                        nc.vector.tensor_max(nacc[:], acc[:], pk[:])
                        acc = nacc
                for q in range(WIN // P):
                    t0 = c0 + n0 + q * P
                    tp = tp_p.tile([P, P], F32, tag="tp")
                    nc.tensor.transpose(
                        tp[:], acc[:, q * P:(q + 1) * P], ident_t[:])
                    tps = tps_p.tile([P, P], F32, tag="tps")
                    nc.scalar.copy(tps[:], tp[:])
                    mxv = mx_p.tile([P, 8], F32, tag="mxv")
                    nc.vector.max(mxv[:], tps[:])
                    mxi = mx_p.tile([P, 8], U32, tag="mxi")
                    nc.vector.max_index(mxi[:], mxv[:], tps[:])
                    nc.gpsimd.dma_start(out[t0:t0 + P, :], mxi[:, 0:1])
                    nc.gpsimd.dma_start(vals[t0:t0 + P, :], mxv[:, 0:2])

    nc.compile()
    return nc


def make_in_maps_screen_b(x: np.ndarray, c16, bias):
    cpad = np.zeros((D, K2), np.float16)
    cpad[:, :K] = c16
    bv = np.full(K2, (257.0 - C0) / BETA, np.float64)  # pads -> u = 257
    bv[:K] = bias
    biasv = (BETA * bv + C0).astype(np.float32).reshape(KT, P).T.copy()
    base = {
        "c": cpad,
        "biasv": biasv,
        "ident": np.eye(P, dtype=np.float32),
    }
    in_maps = []
    for i in range(NCORES):
        m = dict(base)
        m["xT"] = np.ascontiguousarray(
            x[i * NSH:(i + 1) * NSH].T.astype(np.float16))
        in_maps.append(m)
    return in_maps


def _split16(a: np.ndarray, levels: int):
    """Split f32/f64 array into `levels` fp16 rows summing to ~a."""
    rows = []
    rem = a.astype(np.float64)
    for _ in range(levels):
        h = rem.astype(np.float16)
        rows.append(h)
        rem = rem - h.astype(np.float64)
    return rows


def _prep_c(centroids: np.ndarray):
    c = np.ascontiguousarray(centroids, dtype=np.float32)
    c_norm = (c.astype(np.float64) ** 2).sum(axis=0)
    bias = -0.5 * c_norm
    c16 = c.astype(np.float16)
    cl16 = (c - c16.astype(np.float32)).astype(np.float16)
    return c16, cl16, bias


def make_in_maps_screen(x: np.ndarray, c16, bias):
    b_rows = _split16(bias, 2)
    base = {
        "c": c16,
        "bias2": np.stack(b_rows),
        "ones": np.ones((2, P), np.float16),
    }
    in_maps = []
    for i in range(NCORES):
        m = dict(base)
        m["xT"] = np.ascontiguousarray(
            x[i * NSH:(i + 1) * NSH].T.astype(np.float16))
        in_maps.append(m)
    return in_maps


def make_in_maps_exact(x_rows: np.ndarray, c16, cl16, bias, n_rows: int):
    total = n_rows * NCORES
    xp = np.zeros((total, D), dtype=np.float32)
    xp[: len(x_rows)] = x_rows
    xh = xp.astype(np.float16)
    xl = (xp - xh.astype(np.float32)).astype(np.float16)
    b_rows = _split16(bias, 3)
    base = {
        "c": c16,
        "cl": cl16,
        "bias2": np.stack(b_rows),
        "ones": np.ones((3, P), np.float16),
    }
    in_maps = []
    for i in range(NCORES):
        m = dict(base)
        m["xT"] = np.ascontiguousarray(xh[i * n_rows:(i + 1) * n_rows].T)
        m["xTl"] = np.ascontiguousarray(xl[i * n_rows:(i + 1) * n_rows].T)
        in_maps.append(m)
    return in_maps


_NC_CACHE = {}
LAST_RESULTS = []  # (label, BassKernelResults) of the most recent kernel() call


def _cached_nc(key, builder):
    if key not in _NC_CACHE:
        _NC_CACHE[key] = builder()
    return _NC_CACHE[key]


def _run_spmd(nc, in_maps, label):
    kw = {}
    if os.environ.get("KMEANS_TRACE"):
        import shutil

        kw["trace"] = True
        kw["tmpdir"] = os.environ.get("KMEANS_TRACE_DIR", "/tmp/km_trace") + "_" + label
        shutil.rmtree(kw["tmpdir"], ignore_errors=True)
        os.makedirs(kw["tmpdir"], exist_ok=True)
    res = run_bass_kernel_spmd(nc, in_maps, core_ids=list(range(NCORES)), **kw)
    LAST_RESULTS.append((label, res))
    return res


def kernel(x: np.ndarray, centroids: np.ndarray) -> np.ndarray:
    LAST_RESULTS.clear()
    x = np.asarray(x)
    centroids = np.asarray(centroids)
    c16, cl16, bias = _prep_c(centroids)

    # phase 1: fp16 screen with top-2 margins
    th = float(os.environ.get("KMEANS_MARGIN_TH", str(MARGIN_TH_DEFAULT)))
    if os.environ.get("KMEANS_SCREEN", "b") == "b":
        nc1 = _cached_nc(("screen_b", NSH), lambda: build_km_b(NSH))
        res1 = _run_spmd(nc1, make_in_maps_screen_b(x, c16, bias), "phase1")
        p = np.concatenate(
            [res1.results[i]["out"].reshape(NSH) for i in range(NCORES)]
        ).astype(np.int64)
        vals = np.concatenate(
            [res1.results[i]["vals"].reshape(NSH, 2) for i in range(NCORES)])
        bits = np.ascontiguousarray(vals[:, 0]).view(np.uint32)
        kt = (KT - 1) - (bits & 0xF)
        idx = np.minimum(kt * P + p, K - 1).astype(np.int32)
        margin = (vals[:, 0] - vals[:, 1]).astype(np.float64) / BETA
        flagged = np.flatnonzero(margin < th + 2 * (2.0 ** -11) / BETA)
    else:
        nc1 = _cached_nc(("screen", NSH), lambda: build_km(NSH, exact=False))
        res1 = _run_spmd(nc1, make_in_maps_screen(x, c16, bias), "phase1")
        idx = np.concatenate(
            [res1.results[i]["out"].reshape(NSH) for i in range(NCORES)]
        ).astype(np.int32)
        vals = np.concatenate(
            [res1.results[i]["vals"].reshape(NSH, 2) for i in range(NCORES)])
        margin = vals[:, 0] - vals[:, 1]
        flagged = np.flatnonzero(margin < th)

    # phase 2: exact recompute of flagged rows; smallest padded program that
    # covers the count, chunking in the (unexpected) overflow case
    sizes = [128, 256, 512, 1024, 2048]
    per_core = min((s for s in sizes if s * NCORES >= len(flagged)),
                   default=sizes[-1])
    cap = per_core * NCORES
    for s in range(0, len(flagged), cap):
        rows = flagged[s:s + cap]
        nc2 = _cached_nc(("exact", per_core),
                         lambda: build_km(per_core, exact=True))
        res2 = _run_spmd(
            nc2, make_in_maps_exact(x[rows], c16, cl16, bias, per_core),
            "phase2")
        exact_idx = np.concatenate(
            [res2.results[i]["out"].reshape(per_core) for i in range(NCORES)])
        idx[rows] = exact_idx[: len(rows)].astype(np.int32)
    return idx


# revision 12
# speedup vs baseline: 1.8349x; 1.0193x over previous
"""Trainium2 Bass kernel for KMeans assignment (argmin over centroid distances).

Problem: x [131072, 768] f32, centroids [768, 2000] f32
Output:  argmin_k ||x_n - c_k||^2  -> int32 [131072]

Math: argmin_k(||x||^2 - 2 x.c_k + ||c_k||^2) = argmax_k(x.c_k - 0.5||c_k||^2).

Data-parallel over 8 cores (16384 rows each). Two phases:
  phase 1 (screen): single fp16 matmul pass. x is transposed+cast to fp16 on
    the HOST, so the device loop is pure matmul: per 128-row tile, 4 bias
    matmuls (2-row hi/lo fp16, folds -0.5||c||^2) + 24 score matmuls
    accumulate into 4 PSUM banks; ACT evicts to SBUF; DVE max8/find_index8
    produce argmax + top-2 margin. fp16 score error sigma ~0.006, so only
    rows with margin < MARGIN_TH (~0.5%) need exact recompute.
  phase 2 (exact): flagged rows recomputed with fp16 hi/lo 3-pass matmul
    (x.c = xh.ch + xh.cl + xl.ch), bias as 3-row fp16 ladder -> ~1e-5 error.
"""

import os
import sys

for _p in ("/opt/trn_rl_repo",):
    if _p not in sys.path and os.path.isdir(_p):
        sys.path.insert(0, _p)

from contextlib import ExitStack

import numpy as np

import concourse.bass as bass
import concourse.tile as tile
from concourse import bacc, mybir
from concourse.bass_utils import run_bass_kernel_spmd

N, D, K = 131072, 768, 2000
NCORES = 8
NSH = N // NCORES  # 16384 rows per core
P = 128
DT = D // P  # 6 contraction tiles
# score chunks, each within one PSUM bank (<=512 fp32)
KOFF = [0, 512, 1024, 1536]
KW = [512, 512, 512, 464]
NB = 4

F32 = mybir.dt.float32
F16 = mybir.dt.float16
U32 = mybir.dt.uint32

# fp16 screen margin threshold: empirical score err sigma ~0.008, max ~0.05
# on randn data. 0.06 keeps sim misses at 0 while flagging ~0.6% of rows
# (fits the 128-rows/core phase-2 program).
MARGIN_TH_DEFAULT = 0.06


def build_km(n_rows: int, exact: bool):
    """One-pass fp16 screen (exact=False) or 3-pass fp16 hi/lo (exact=True).

    Inputs are host-prepared: xT (and xTl for exact) is x transposed to
    [D, n_rows] fp16 so no on-device transpose/cast is needed; c in fp16
    (plus cl residual for exact); bias ladder rows fold -0.5||c||^2 into the
    PSUM accumulation via a tiny extra matmul.
    """
    assert n_rows % P == 0
    nt = n_rows // P
    CHUNK = min(2048, n_rows)
    assert n_rows % CHUNK == 0
    nchunks = n_rows // CHUNK
    tpc = CHUNK // P
    NBIAS = 3 if exact else 2

    nc = bacc.Bacc("TRN2", target_bir_lowering=False, debug=False)

    xT = nc.dram_tensor("xT", [D, n_rows], F16, kind="ExternalInput").ap()
    c_in = nc.dram_tensor("c", [D, K], F16, kind="ExternalInput").ap()
    bias2 = nc.dram_tensor("bias2", [NBIAS, K], F16, kind="ExternalInput").ap()
    ones = nc.dram_tensor("ones", [NBIAS, P], F16, kind="ExternalInput").ap()
    if exact:
        xTl = nc.dram_tensor("xTl", [D, n_rows], F16, kind="ExternalInput").ap()
        cl_in = nc.dram_tensor("cl", [D, K], F16, kind="ExternalInput").ap()
    out = nc.dram_tensor("out", [n_rows, 1], U32, kind="ExternalOutput").ap()
    vals = nc.dram_tensor("vals", [n_rows, 2], F32, kind="ExternalOutput").ap()

    with tile.TileContext(nc) as tc, ExitStack() as ctx:
        const = ctx.enter_context(tc.tile_pool(name="const", bufs=1))
        xsl_p = ctx.enter_context(tc.tile_pool(name="xsl", bufs=2))
        sc_p = ctx.enter_context(tc.tile_pool(name="sc", bufs=2, space="PSUM"))
        ss_p = ctx.enter_context(tc.tile_pool(name="ss", bufs=2))
        mx_p = ctx.enter_context(tc.tile_pool(name="mx", bufs=4))

        # c (and cl) loads split across the three DMA-capable queues
        dma_engines = [nc.sync, nc.scalar, nc.gpsimd]
        c3 = c_in.rearrange("(t p) k -> p t k", p=P)
        c_all = const.tile([P, DT, K], F16, tag="c_all")
        for d in range(DT):
            dma_engines[d % 3].dma_start(c_all[:, d, :], c3[:, d, :])
        c_tiles = [c_all[:, d, :] for d in range(DT)]
        if exact:
            cl3 = cl_in.rearrange("(t p) k -> p t k", p=P)
            cl_all = const.tile([P, DT, K], F16, tag="cl_all")
            for d in range(DT):
                dma_engines[(d + 1) % 3].dma_start(cl_all[:, d, :], cl3[:, d, :])
            cl_tiles = [cl_all[:, d, :] for d in range(DT)]
        bias_t = const.tile([NBIAS, K], F16, tag="bias2")
        nc.scalar.dma_start(bias_t[:], bias2[:, :])
        ones_t = const.tile([NBIAS, P], F16, tag="ones")
        nc.scalar.dma_start(ones_t[:], ones[:, :])

        xT3 = xT.rearrange("(t p) n -> p t n", p=P)
        if exact:
            xTl3 = xTl.rearrange("(t p) n -> p t n", p=P)

        for ch in range(nchunks):
            c0 = ch * CHUNK
            slab = xsl_p.tile([P, DT, CHUNK], F16, tag="sl")
            nc.sync.dma_start(slab[:], xT3[:, :, c0:c0 + CHUNK])
            slabs = [slab[:, d, :] for d in range(DT)]
            if exact:
                slab_l = xsl_p.tile([P, DT, CHUNK], F16, tag="sll")
                nc.sync.dma_start(slab_l[:], xTl3[:, :, c0:c0 + CHUNK])
                slabs_l = [slab_l[:, d, :] for d in range(DT)]

            for tt in range(tpc):
                off = tt * P
                banks = [
                    sc_p.tile([P, KW[b]], F32, tag=f"b{b}", name=f"bank{b}")
                    for b in range(NB)
                ]
                for b in range(NB):
                    nc.tensor.matmul(
                        banks[b][:], ones_t[:],
                        bias_t[:, KOFF[b]:KOFF[b] + KW[b]],
                        start=True, stop=False)
                if exact:
                    # xh.ch + xh.cl + xl.ch
                    passes = [(slabs, c_tiles), (slabs, cl_tiles),
                              (slabs_l, c_tiles)]
                else:
                    passes = [(slabs, c_tiles)]
                np_ = len(passes)
                for pi, (xs, cs) in enumerate(passes):
                    for d in range(DT):
                        last = pi == np_ - 1 and d == DT - 1
                        for b in range(NB):
                            nc.tensor.matmul(
                                banks[b][:], xs[d][:, off:off + P],
                                cs[d][:, KOFF[b]:KOFF[b] + KW[b]],
                                start=False, stop=last)

                ss = ss_p.tile([P, K], F32)
                for b in range(NB):
                    nc.scalar.copy(ss[:, KOFF[b]:KOFF[b] + KW[b]], banks[b][:])

                t = ch * tpc + tt
                mxv = mx_p.tile([P, 8], F32, tag="mxv")
                nc.vector.max(mxv[:], ss[:])
                mxi = mx_p.tile([P, 8], U32, tag="mxi")
                nc.vector.max_index(mxi[:], mxv[:], ss[:])
                nc.gpsimd.dma_start(out[t * P:(t + 1) * P, :], mxi[:, 0:1])
                nc.gpsimd.dma_start(vals[t * P:(t + 1) * P, :], mxv[:, 0:2])

    nc.compile()
    return nc


#  Orientation-B screen: scores computed k-on-partitions so the -0.5||c||^2
#  bias is a per-partition ACT bias (no bias matmuls at all). Each kt-tile's
#  [128k, 512n] PSUM block is evicted by ACT as u = BETA*s + bias_k + C0,
#  pinned inside the f32 octave [256, 512) so ulp is constant 2^-15. DVE
#  packs the kt-tile id into the low 4 mantissa bits ((u & ~0xF) | (15-kt))
#  and chain-maxes the 16 kt-tiles; PE transposes the 128-wide survivor per
#  row-tile; max8 + find_index8 (128-wide) give the winning partition p and
#  packed value. Host decodes k = (15 - (bits & 0xF))*128 + p. Ties at the
#  2^-11 quantum prefer smaller k (matching first-argmin) and near-ties are
#  margin-flagged for phase 2 anyway.
BETA = 0.25
C0 = 448.0
K2 = 2048  # K padded to 16 full kt-tiles; pads get bias -> u=257, never win
KT = K2 // P
WIN = 512  # rows per score window (one PSUM bank of [128k, 512n])


def build_km_b(n_rows: int):
    assert n_rows % P == 0
    CHUNK = min(512, n_rows)
    assert n_rows % CHUNK == 0
    nchunks = n_rows // CHUNK
    wpc = CHUNK // WIN

    nc = bacc.Bacc("TRN2", target_bir_lowering=False, debug=False)

    xT = nc.dram_tensor("xT", [D, n_rows], F16, kind="ExternalInput").ap()
    c_in = nc.dram_tensor("c", [D, K2], F16, kind="ExternalInput").ap()
    biasv = nc.dram_tensor("biasv", [P, KT], F32, kind="ExternalInput").ap()
    ident = nc.dram_tensor("ident", [P, P], F32, kind="ExternalInput").ap()
    out = nc.dram_tensor("out", [n_rows, 1], U32, kind="ExternalOutput").ap()
    vals = nc.dram_tensor("vals", [n_rows, 2], F32, kind="ExternalOutput").ap()

    with tile.TileContext(nc) as tc, ExitStack() as ctx:
        const = ctx.enter_context(tc.tile_pool(name="const", bufs=1))
        xsl_p = ctx.enter_context(tc.tile_pool(name="xsl", bufs=2))
        ps_p = ctx.enter_context(tc.tile_pool(name="ps", bufs=4, space="PSUM"))
        tp_p = ctx.enter_context(tc.tile_pool(name="tp", bufs=2, space="PSUM"))
        ev_p = ctx.enter_context(tc.tile_pool(name="ev", bufs=3))
        pk_p = ctx.enter_context(tc.tile_pool(name="pk", bufs=2))
        acc_p = ctx.enter_context(tc.tile_pool(name="acc", bufs=2))
        tps_p = ctx.enter_context(tc.tile_pool(name="tps", bufs=2))
        mx_p = ctx.enter_context(tc.tile_pool(name="mx", bufs=4))

        c3 = c_in.rearrange("(t p) k -> p t k", p=P)
        c_all = const.tile([P, DT, K2], F16, tag="c_all")
        dma_engines = [nc.sync, nc.scalar, nc.gpsimd]
        for d in range(DT):
            dma_engines[d % 3].dma_start(c_all[:, d, :], c3[:, d, :])
        biasv_t = const.tile([P, KT], F32, tag="biasv")
        nc.scalar.dma_start(biasv_t[:], biasv[:, :])
        ident_t = const.tile([P, P], F32, tag="ident")
        nc.scalar.dma_start(ident_t[:], ident[:, :])

        xT3 = xT.rearrange("(t p) n -> p t n", p=P)
        ID = mybir.ActivationFunctionType.Identity
        for ch in range(nchunks):
            c0 = ch * CHUNK
            slab = xsl_p.tile([P, DT, CHUNK], F16, tag="sl")
            nc.sync.dma_start(slab[:], xT3[:, :, c0:c0 + CHUNK])
            for w in range(wpc):
                n0 = w * WIN
                pks = []
                for kt in range(KT):
                    ps = ps_p.tile([P, WIN], F32, tag="ps", name="psb")
                    for d in range(DT):
                        nc.tensor.matmul(
                            ps[:], c_all[:, d, kt * P:(kt + 1) * P],
                            slab[:, d, n0:n0 + WIN],
                            start=(d == 0), stop=(d == DT - 1))
                    sb = ev_p.tile([P, WIN], F32, tag="ev")
                    nc.scalar.activation(sb[:], ps[:], ID,
                                         bias=biasv_t[:, kt:kt + 1],
                                         scale=BETA)
                    pk = pk_p.tile([P, WIN], F32, tag=f"pk{kt % 4}")
                    nc.vector.tensor_scalar(
                        pk.bitcast(U32)[:], sb.bitcast(U32)[:],
                        0xFFFFFFF0, KT - 1 - kt,
                        op0=mybir.AluOpType.bitwise_and,
                        op1=mybir.AluOpType.bitwise_or)
                    pks.append(pk)
                # pairwise tree: depth 4 instead of a 15-deep serial chain,
                # so the last window's reduction tail is short
                level = 1
                while len(pks) > 1:
                    nxt = []
                    for i in range(0, len(pks) - 1, 2):
                        t = acc_p.tile([P, WIN], F32,
                                       tag=f"l{level}.{i % 4}")
                        nc.vector.tensor_max(t[:], pks[i][:], pks[i + 1][:])
                        nxt.append(t)
                    if len(pks) % 2:
                        nxt.append(pks[-1])
                    pks = nxt
                    level += 1
                acc = pks[0]
                for q in range(WIN // P):
                    t0 = c0 + n0 + q * P
                    tp = tp_p.tile([P, P], F32, tag="tp")
                    nc.tensor.transpose(
                        tp[:], acc[:, q * P:(q + 1) * P], ident_t[:])
                    tps = tps_p.tile([P, P], F32, tag="tps")
                    nc.scalar.copy(tps[:], tp[:])
                    mxv = mx_p.tile([P, 8], F32, tag="mxv")
                    nc.vector.max(mxv[:], tps[:])
                    mxi = mx_p.tile([P, 8], U32, tag="mxi")
                    nc.vector.max_index(mxi[:], mxv[:], tps[:])
                    nc.gpsimd.dma_start(out[t0:t0 + P, :], mxi[:, 0:1])
                    nc.gpsimd.dma_start(vals[t0:t0 + P, :], mxv[:, 0:2])

    nc.compile()
    return nc


def make_in_maps_screen_b(x: np.ndarray, c16, bias):
    cpad = np.zeros((D, K2), np.float16)
    cpad[:, :K] = c16
    bv = np.full(K2, (257.0 - C0) / BETA, np.float64)  # pads -> u = 257
    bv[:K] = bias
    biasv = (BETA * bv + C0).astype(np.float32).reshape(KT, P).T.copy()
    base = {
        "c": cpad,
        "biasv": biasv,
        "ident": np.eye(P, dtype=np.float32),
    }
    in_maps = []
    for i in range(NCORES):
        m = dict(base)
        m["xT"] = np.ascontiguousarray(
            x[i * NSH:(i + 1) * NSH].T.astype(np.float16))
        in_maps.append(m)
    return in_maps


def _split16(a: np.ndarray, levels: int):
    """Split f32/f64 array into `levels` fp16 rows summing to ~a."""
    rows = []
    rem = a.astype(np.float64)
    for _ in range(levels):
        h = rem.astype(np.float16)
        rows.append(h)
        rem = rem - h.astype(np.float64)
    return rows


def _prep_c(centroids: np.ndarray):
    c = np.ascontiguousarray(centroids, dtype=np.float32)
    c_norm = (c.astype(np.float64) ** 2).sum(axis=0)
    bias = -0.5 * c_norm
    c16 = c.astype(np.float16)
    cl16 = (c - c16.astype(np.float32)).astype(np.float16)
    return c16, cl16, bias


def make_in_maps_screen(x: np.ndarray, c16, bias):
    b_rows = _split16(bias, 2)
    base = {
        "c": c16,
        "bias2": np.stack(b_rows),
        "ones": np.ones((2, P), np.float16),
    }
    in_maps = []
    for i in range(NCORES):
        m = dict(base)
        m["xT"] = np.ascontiguousarray(
            x[i * NSH:(i + 1) * NSH].T.astype(np.float16))
        in_maps.append(m)
    return in_maps


def make_in_maps_exact(x_rows: np.ndarray, c16, cl16, bias, n_rows: int):
    total = n_rows * NCORES
    xp = np.zeros((total, D), dtype=np.float32)
    xp[: len(x_rows)] = x_rows
    xh = xp.astype(np.float16)
    xl = (xp - xh.astype(np.float32)).astype(np.float16)
    b_rows = _split16(bias, 3)
    base = {
        "c": c16,
        "cl": cl16,
        "bias2": np.stack(b_rows),
        "ones": np.ones((3, P), np.float16),
    }
    in_maps = []
    for i in range(NCORES):
        m = dict(base)
        m["xT"] = np.ascontiguousarray(xh[i * n_rows:(i + 1) * n_rows].T)
        m["xTl"] = np.ascontiguousarray(xl[i * n_rows:(i + 1) * n_rows].T)
        in_maps.append(m)
    return in_maps


_NC_CACHE = {}
LAST_RESULTS = []  # (label, BassKernelResults) of the most recent kernel() call


def _cached_nc(key, builder):
    if key not in _NC_CACHE:
        _NC_CACHE[key] = builder()
    return _NC_CACHE[key]


def _run_spmd(nc, in_maps, label):
    kw = {}
    if os.environ.get("KMEANS_TRACE"):
        import shutil

        kw["trace"] = True
        kw["tmpdir"] = os.environ.get("KMEANS_TRACE_DIR", "/tmp/km_trace") + "_" + label
        shutil.rmtree(kw["tmpdir"], ignore_errors=True)
        os.makedirs(kw["tmpdir"], exist_ok=True)
    res = run_bass_kernel_spmd(nc, in_maps, core_ids=list(range(NCORES)), **kw)
    LAST_RESULTS.append((label, res))
    return res


def kernel(x: np.ndarray, centroids: np.ndarray) -> np.ndarray:
    LAST_RESULTS.clear()
    x = np.asarray(x)
    centroids = np.asarray(centroids)
    c16, cl16, bias = _prep_c(centroids)

    # phase 1: fp16 screen with top-2 margins
    th = float(os.environ.get("KMEANS_MARGIN_TH", str(MARGIN_TH_DEFAULT)))
    if os.environ.get("KMEANS_SCREEN", "b") == "b":
        nc1 = _cached_nc(("screen_b", NSH), lambda: build_km_b(NSH))
        res1 = _run_spmd(nc1, make_in_maps_screen_b(x, c16, bias), "phase1")
        p = np.concatenate(
            [res1.results[i]["out"].reshape(NSH) for i in range(NCORES)]
        ).astype(np.int64)
        vals = np.concatenate(
            [res1.results[i]["vals"].reshape(NSH, 2) for i in range(NCORES)])
        bits = np.ascontiguousarray(vals[:, 0]).view(np.uint32)
        kt = (KT - 1) - (bits & 0xF)
        idx = np.minimum(kt * P + p, K - 1).astype(np.int32)
        margin = (vals[:, 0] - vals[:, 1]).astype(np.float64) / BETA
        flagged = np.flatnonzero(margin < th + 2 * (2.0 ** -11) / BETA)
    else:
        nc1 = _cached_nc(("screen", NSH), lambda: build_km(NSH, exact=False))
        res1 = _run_spmd(nc1, make_in_maps_screen(x, c16, bias), "phase1")
        idx = np.concatenate(
            [res1.results[i]["out"].reshape(NSH) for i in range(NCORES)]
        ).astype(np.int32)
        vals = np.concatenate(
            [res1.results[i]["vals"].reshape(NSH, 2) for i in range(NCORES)])
        margin = vals[:, 0] - vals[:, 1]
        flagged = np.flatnonzero(margin < th)

    # phase 2: exact recompute of flagged rows; smallest padded program that
    # covers the count, chunking in the (unexpected) overflow case
    sizes = [128, 256, 512, 1024, 2048]
    per_core = min((s for s in sizes if s * NCORES >= len(flagged)),
                   default=sizes[-1])
    cap = per_core * NCORES
    for s in range(0, len(flagged), cap):
        rows = flagged[s:s + cap]
        nc2 = _cached_nc(("exact", per_core),
                         lambda: build_km(per_core, exact=True))
        res2 = _run_spmd(
            nc2, make_in_maps_exact(x[rows], c16, cl16, bias, per_core),
            "phase2")
        exact_idx = np.concatenate(
            [res2.results[i]["out"].reshape(per_core) for i in range(NCORES)])
        idx[rows] = exact_idx[: len(rows)].astype(np.int32)
    return idx
